# revision 20
# baseline (speedup 1.0000x reference)
"""Trainium2 Bass kernel for nn_MiddleOut (gnn_message_passing).

Math (reference):
    out[b,r] = mean_p[ m[b,p] * (my@Wm.T + bias + peer[b,p]@Wp.T + m[b,p]*wm)[r] ]
Collapses to (P = #peers):
    s1[b] = sum_p m[b,p];  s2[b] = sum_p m[b,p]^2
    z[b,l] = sum_p m[b,p] * peer[b,p,l]
    out = s1*(my@Wm.T)/P + z@(Wp.T/P) + s1*(bias/P) + s2*(wm/P)

Sharding: pure data parallel over batch across 8 cores (2048 rows each,
16 tiles of 128).

v4 design (memory regime: minimize HBM bytes, keep every engine under the
~55us DMA floor):
  - peer tile host-cast to fp8 e3m4 and PE-band-permuted [(b4,p), g, l+2];
    the two extra moving columns per group are [1.0 | m] so the 32-matmul
    band chain also produces s1 (col 256) and s2 (col 257) for free.
  - z-chain on PE for ALL tiles: block-diag fp8 metric band stationary
    (4x FWL weight loads; rewritten per tile by 4 DVE strided cast-copies,
    ping-pong x3, memset lazily) vs fp8 moving, accumulating
    psum_z[b, 0:258] over 32 groups.
  - epilogue per tile: ACT evac psum_z -> x_sb f32; two PE transposes into
    one [128,256] PSUM tile; ONE ACT cast to f32r; 2 f32r matmuls
    (z @ Wp.T/P) -> psum_rest; 2 bf16 matmuls with host-pretransposed myT
    (straight from the meta tile) -> psum_my; ACT evac psum_my scaled by
    s1; the rank-1 s1*bias/P + s2*wm/P terms via two DVE stt ops against
    host-replicated rows; DVE add -> out bf16.
  - startup: first x/meta DMAs issued before weights; x in two half-tile
    DMAs so the first chain starts earlier.
  - my/metrics arrive in one packed bf16 meta tile; out is written bf16
    and upcast on host; 1/P folded into the host-packed weights.
"""

import ml_dtypes
import numpy as np

import concourse.bass as bass
import concourse.mybir as mybir
import concourse.tile as tile
from concourse import bacc
from concourse.bass_utils import run_bass_kernel_spmd

F32 = mybir.dt.float32
F32R = mybir.dt.float32r
BF = mybir.dt.bfloat16
F8 = mybir.dt.float8e3

B, P, L, R = 16384, 32, 256, 256
N_CORES = 8
BC = B // N_CORES          # 2048 batches per core
TILE_B = 128               # batches per SBUF tile
NT = BC // TILE_B          # 16 tiles
G = TILE_B // 4            # 32 groups of 4 batches
LX = L + 2                 # moving cols per group: [x | 1.0 | m]
MC = G + 2 * TILE_B        # meta cols: [mt | myT0 | myT1]

_cache = {}


def build_bass(nt=NT, num_devices=N_CORES):
    bc = nt * TILE_B
    nc = bacc.Bacc(
        "TRN2", target_bir_lowering=False, debug=False, num_devices=num_devices
    )

    x_d = nc.dram_tensor("x", [nt, TILE_B, G, LX], F8, kind="ExternalInput")
    # meta packs [mt | myT0 | myT1] per tile (all bf16):
    #   cols 0:G      mt   metric band source, partition=(b4,p), col=g
    #   cols 32:160   myT chunk0: partition=l, col=b   (l in [0,128))
    #   cols 160:288  myT chunk1: partition=l-128, col=b
    meta_d = nc.dram_tensor("meta", [nt, TILE_B, MC], BF, kind="ExternalInput")
    wr_d = nc.dram_tensor("wr", [2, TILE_B, R], BF, kind="ExternalInput")
    wb_d = nc.dram_tensor("wb", [2, TILE_B, R], BF, kind="ExternalInput")
    # row-replicated [bias/P ; wm/P] for the DVE rank-1 terms
    wf_d = nc.dram_tensor("wf", [2, TILE_B, R], F32, kind="ExternalInput")
    out_d = nc.dram_tensor("out", [bc, R], BF, kind="ExternalOutput")

    with TileCtx(nc) as (tc, ctx):
        singles = ctx.enter_context(tc.tile_pool(name="singles", bufs=1))
        xp = ctx.enter_context(tc.tile_pool(name="xp", bufs=6))
        small = ctx.enter_context(tc.tile_pool(name="small", bufs=6))
        xtp = ctx.enter_context(tc.tile_pool(name="xtp", bufs=3))
        psz = ctx.enter_context(tc.tile_pool(name="psz", bufs=3, space="PSUM"))
        psr = ctx.enter_context(tc.tile_pool(name="psr", bufs=2, space="PSUM"))
        psm = ctx.enter_context(tc.tile_pool(name="psm", bufs=2, space="PSUM"))

        metas = [None] * nt

        def load_meta(t):
            meta = small.tile([TILE_B, MC], BF, tag="meta")
            nc.scalar.dma_start(out=meta, in_=meta_d[t])
            metas[t] = meta

        # critical-path loads first: tile 0 x + meta
        x_tiles = [None] * nt

        def load_x(t):
            x_t = xp.tile([TILE_B, G, LX], F8, tag="x_t")
            nc.sync.dma_start(out=x_t[:, 0:G // 2, :], in_=x_d[t, :, 0:G // 2, :])
            nc.sync.dma_start(out=x_t[:, G // 2:, :], in_=x_d[t, :, G // 2:, :])
            x_tiles[t] = x_t

        load_x(0)
        load_meta(0)

        wr_sb = singles.tile([TILE_B, 2, R], BF)
        nc.sync.dma_start(out=wr_sb, in_=wr_d.rearrange("k p r -> p k r"))
        wb_sb = singles.tile([TILE_B, 2, R], BF)
        nc.sync.dma_start(out=wb_sb, in_=wb_d.rearrange("k p r -> p k r"))
        wf_sb = singles.tile([TILE_B, 2, R], F32)
        nc.sync.dma_start(out=wf_sb, in_=wf_d.rearrange("k p r -> p k r"))

        # Ping-pong block-diagonal stationaries for the PE peer-reduce.
        # s[:, g, :] is [128, 128]: column 4g+b4 holds m[g*4+b4, p] at rows
        # (b4, p); zeros written lazily, diagonal band rewritten per tile.
        s_tiles = [
            singles.tile([TILE_B, G, TILE_B], F8, tag=f"s{i}", name=f"s_{i}")
            for i in range(3)
        ]

        def write_band(t):
            s_all = s_tiles[t % 3]
            if t < 3:
                nc.vector.memset(s_all, 0.0)
            m_t = metas[t][:, 0:G]
            for b4 in range(4):
                view = s_all[b4 * P:(b4 + 1) * P, :, :]
                out_ap = bass.AP(
                    tensor=view.tensor, offset=view.offset + b4,
                    ap=[view.ap[0], [TILE_B + 4, G]],
                )
                nc.vector.tensor_copy(
                    out=out_ap, in_=m_t[b4 * P:(b4 + 1) * P, :],
                )

        write_band(0)
        for t in range(nt):
            if t > 0:
                load_x(t)
            x_t = x_tiles[t]
            if t + 1 < nt:
                load_meta(t + 1)

            # ---- z-chain: psum_z[b, 0:256]=z, [256]=s1, [257]=s2 ----
            s_all = s_tiles[t % 3]
            psum_z = psz.tile([TILE_B, LX], F32, tag="psum_z")
            for g in range(G):
                nc.tensor.matmul(
                    out=psum_z,
                    lhsT=s_all[:, g, :],
                    rhs=x_t[:, g, :],
                    start=(g == 0),
                    stop=(g == G - 1),
                )

            # band for the next tile while PE runs this epilogue
            if t + 1 < nt:
                write_band(t + 1)

            # ---- epilogue ----
            # z chunks to bf16 for the xbar DMA-transpose; s1/s2 kept f32
            x_sb = small.tile([TILE_B, L], BF, tag="x_sb")
            nc.scalar.copy(out=x_sb, in_=psum_z[:, 0:L])
            s12_sb = small.tile([TILE_B, 2], F32, tag="s12_sb")
            nc.vector.tensor_copy(out=s12_sb, in_=psum_z[:, L:LX])
            s1 = s12_sb[:, 0:1]
            s2 = s12_sb[:, 1:2]

            meta = metas[t]
            psum_my = psm.tile([TILE_B, R], F32, tag="psum_my")
            nc.tensor.matmul(
                out=psum_my, lhsT=meta[:, G:G + TILE_B], rhs=wb_sb[:, 0, :],
                start=True, stop=False,
            )
            nc.tensor.matmul(
                out=psum_my, lhsT=meta[:, G + TILE_B:MC], rhs=wb_sb[:, 1, :],
                start=False, stop=True,
            )

            # transpose z chunks via the DMA xbar (SBUF->SBUF, bf16)
            zt0 = xtp.tile([TILE_B, TILE_B], BF, tag="zt0")
            nc.scalar.dma_start_transpose(out=zt0, in_=x_sb[:, 0:TILE_B])
            zt1 = xtp.tile([TILE_B, TILE_B], BF, tag="zt1")
            nc.scalar.dma_start_transpose(
                out=zt1, in_=x_sb[:, TILE_B:2 * TILE_B]
            )

            # psum_rest = z @ Wp.T/P
            psum_rest = psr.tile([TILE_B, R], F32, tag="psum_rest")
            nc.tensor.matmul(
                out=psum_rest, lhsT=zt0, rhs=wr_sb[:, 0, :],
                start=True, stop=False,
            )
            nc.tensor.matmul(
                out=psum_rest, lhsT=zt1, rhs=wr_sb[:, 1, :],
                start=False, stop=True,
            )

            # out = s1*psum_my + psum_rest + s1*(bias/P) + s2*(wm/P)
            # rank-1 terms on GPSIMD (SBUF-only), PSUM add on DVE
            my_sb = small.tile([TILE_B, R], F32, tag="my_sb")
            nc.scalar.mul(out=my_sb, in_=psum_my, mul=s1)
            t1 = small.tile([TILE_B, R], F32, tag="t1")
            nc.vector.scalar_tensor_tensor(
                out=t1, in0=wf_sb[:, 0, :], scalar=s1, in1=my_sb,
                op0=mybir.AluOpType.mult, op1=mybir.AluOpType.add,
            )
            t2 = small.tile([TILE_B, R], F32, tag="t2")
            nc.vector.scalar_tensor_tensor(
                out=t2, in0=wf_sb[:, 1, :], scalar=s2, in1=t1,
                op0=mybir.AluOpType.mult, op1=mybir.AluOpType.add,
            )
            out_sb = small.tile([TILE_B, R], BF, tag="out_sb")
            nc.vector.tensor_add(out_sb, t2, psum_rest)
            nc.scalar.dma_start(
                out=out_d[t * TILE_B:(t + 1) * TILE_B, :], in_=out_sb
            )

    nc.compile()
    return nc


class TileCtx:
    """with TileCtx(nc) as (tc, ctx): — TileContext plus an ExitStack."""

    def __init__(self, nc):
        from contextlib import ExitStack
        self.tc = tile.TileContext(nc)
        self.ctx = ExitStack()

    def __enter__(self):
        return self.tc.__enter__(), self.ctx.__enter__()

    def __exit__(self, *a):
        self.ctx.__exit__(*a)
        return self.tc.__exit__(*a)


def prep_inputs(my_latent, peer_latents, peer_metrics, W, b):
    """Host-side shard + layout prep (dtype casts and weight packing only)."""
    wr = np.zeros((2, TILE_B, R), dtype=ml_dtypes.bfloat16)
    wpt = np.ascontiguousarray(W[:, L:2 * L].T) / P       # [256, 256] Wp.T/P
    wr[0] = wpt[0:TILE_B].astype(ml_dtypes.bfloat16)
    wr[1] = wpt[TILE_B:2 * TILE_B].astype(ml_dtypes.bfloat16)
    wb = np.zeros((2, TILE_B, R), dtype=ml_dtypes.bfloat16)
    wmt = np.ascontiguousarray(W[:, 0:L].T) / P           # [256, 256] Wm.T/P
    wb[0] = wmt[0:TILE_B].astype(ml_dtypes.bfloat16)
    wb[1] = wmt[TILE_B:2 * TILE_B].astype(ml_dtypes.bfloat16)
    wf = np.empty((2, TILE_B, R), dtype=np.float32)
    wf[0] = np.broadcast_to(b / P, (TILE_B, R))           # pairs with s1
    wf[1] = np.broadcast_to(W[:, 2 * L] / P, (TILE_B, R))  # wm/P, with s2

    in_maps = []
    for c in range(N_CORES):
        sl = slice(c * BC, (c + 1) * BC)
        # PE band layout [(b4,p), g, l] + the two extra moving cols
        perm = peer_latents[sl].reshape(NT, G, 4, P, L).transpose(
            0, 2, 3, 1, 4)                                # [NT, 4, P, G, L]
        mt = peer_metrics[sl].reshape(NT, G, 4, P).transpose(
            0, 2, 3, 1).reshape(NT, TILE_B, G)            # [NT, (b4,p), G]
        xc = np.empty((NT, TILE_B, G, LX), dtype=ml_dtypes.float8_e3m4)
        xc[:, :, :, 0:L] = perm.reshape(NT, TILE_B, G, L).astype(
            ml_dtypes.float8_e3m4)
        xc[:, :, :, L] = ml_dtypes.float8_e3m4(1.0)
        xc[:, :, :, L + 1] = mt.astype(ml_dtypes.float8_e3m4)

        meta = np.empty((NT, TILE_B, MC), dtype=ml_dtypes.bfloat16)
        meta[:, :, 0:G] = mt.astype(ml_dtypes.bfloat16)
        myt = my_latent[sl].reshape(NT, TILE_B, L).transpose(0, 2, 1).astype(
            ml_dtypes.bfloat16)                           # [NT, l, b]
        meta[:, :, G:G + TILE_B] = myt[:, 0:TILE_B, :]
        meta[:, :, G + TILE_B:] = myt[:, TILE_B:2 * TILE_B, :]
        in_maps.append({
            "x": xc,
            "meta": meta,
            "wr": wr,
            "wb": wb,
            "wf": wf,
        })
    return in_maps


def run(my_latent, peer_latents, peer_metrics, W, b, trace=False, **kw):
    if "nc" not in _cache:
        _cache["nc"] = build_bass()
    nc = _cache["nc"]
    in_maps = prep_inputs(
        np.asarray(my_latent, dtype=np.float32),
        np.asarray(peer_latents, dtype=np.float32),
        np.asarray(peer_metrics, dtype=np.float32),
        np.asarray(W, dtype=np.float32),
        np.asarray(b, dtype=np.float32),
    )
    res = run_bass_kernel_spmd(
        nc, in_maps, core_ids=list(range(N_CORES)), trace=trace, **kw
    )
    out = np.concatenate(
        [np.asarray(r["out"]).astype(np.float32) for r in res.results], axis=0
    )
    return out, res


def kernel(my_latent, peer_latents, peer_metrics, W, b):
    out, _ = run(my_latent, peer_latents, peer_metrics, W, b)
    return out


# revision 28
# speedup vs baseline: 1.9341x; 1.9341x over previous
"""Trainium2 Bass kernel for nn_MiddleOut (gnn_message_passing).

Math (reference):
    out[b,r] = mean_p[ m[b,p] * (my@Wm.T + bias + peer[b,p]@Wp.T + m[b,p]*wm)[r] ]
Collapses to (P = #peers):
    s1[b] = sum_p m[b,p];  s2[b] = sum_p m[b,p]^2
    z[b,l] = sum_p m[b,p] * peer[b,p,l]
    out = s1*(my@Wm.T)/P + z@(Wp.T/P) + s1*(bias/P) + s2*(wm/P)

Sharding: pure data parallel over batch across 8 cores (2048 rows each,
16 tiles of 128).

v4 design (memory regime: minimize HBM bytes, keep every engine under the
~55us DMA floor):
  - peer tile host-cast to fp8 e3m4 and PE-band-permuted [(b4,p), g, l+2];
    the two extra moving columns per group are [1.0 | m] so the 32-matmul
    band chain also produces s1 (col 256) and s2 (col 257) for free.
  - z-chain on PE for ALL tiles: block-diag fp8 metric band stationary
    (4x FWL weight loads; rewritten per tile by 4 DVE strided cast-copies,
    ping-pong x3, memset lazily) vs fp8 moving, accumulating
    psum_z[b, 0:258] over 32 groups.
  - epilogue per tile: ACT evac psum_z -> x_sb f32; two PE transposes into
    one [128,256] PSUM tile; ONE ACT cast to f32r; 2 f32r matmuls
    (z @ Wp.T/P) -> psum_rest; 2 bf16 matmuls with host-pretransposed myT
    (straight from the meta tile) -> psum_my; ACT evac psum_my scaled by
    s1; the rank-1 s1*bias/P + s2*wm/P terms via two DVE stt ops against
    host-replicated rows; DVE add -> out bf16.
  - startup: first x/meta DMAs issued before weights; x in two half-tile
    DMAs so the first chain starts earlier.
  - my/metrics arrive in one packed bf16 meta tile; out is written bf16
    and upcast on host; 1/P folded into the host-packed weights.
"""

import ml_dtypes
import numpy as np

import concourse.bass as bass
import concourse.mybir as mybir
import concourse.tile as tile
from concourse import bacc
from concourse.bass_utils import run_bass_kernel_spmd

F32 = mybir.dt.float32
F32R = mybir.dt.float32r
BF = mybir.dt.bfloat16
F8 = mybir.dt.float8e3

B, P, L, R = 16384, 32, 256, 256
N_CORES = 8
BC = B // N_CORES          # 2048 batches per core
TILE_B = 128               # batches per SBUF tile
NT = BC // TILE_B          # 16 tiles
G = TILE_B // 4            # 32 groups of 4 batches
LX = L + 2                 # moving cols per group: [x | 1.0 | m]
MC = G + 2 * TILE_B        # meta cols: [mt | myT0 | myT1]

_cache = {}


def build_bass(nt=NT, num_devices=N_CORES):
    bc = nt * TILE_B
    nc = bacc.Bacc(
        "TRN2", target_bir_lowering=False, debug=False, num_devices=num_devices
    )

    x_d = nc.dram_tensor("x", [nt, TILE_B, G, LX], F8, kind="ExternalInput")
    # meta packs [mt | myT0 | myT1] per tile (all bf16):
    #   cols 0:G      mt   metric band source, partition=(b4,p), col=g
    #   cols 32:160   myT chunk0: partition=l, col=b   (l in [0,128))
    #   cols 160:288  myT chunk1: partition=l-128, col=b
    meta_d = nc.dram_tensor("meta", [nt, TILE_B, MC], BF, kind="ExternalInput")
    wr_d = nc.dram_tensor("wr", [2, TILE_B, R], F32R, kind="ExternalInput")
    wb_d = nc.dram_tensor("wb", [2, TILE_B, R], BF, kind="ExternalInput")
    # row-replicated [bias/P ; wm/P] for the DVE rank-1 terms
    wf_d = nc.dram_tensor("wf", [2, TILE_B, R], F32, kind="ExternalInput")
    id_d = nc.dram_tensor("ident", [TILE_B, TILE_B], F32, kind="ExternalInput")
    sz_d = nc.dram_tensor("sz", [TILE_B, G, TILE_B], F8, kind="ExternalInput")
    out_d = nc.dram_tensor("out", [bc, R], BF, kind="ExternalOutput")

    with TileCtx(nc) as (tc, ctx):
        singles = ctx.enter_context(tc.tile_pool(name="singles", bufs=1))
        xp = ctx.enter_context(tc.tile_pool(name="xp", bufs=6))
        small = ctx.enter_context(tc.tile_pool(name="small", bufs=6))
        xtp = ctx.enter_context(tc.tile_pool(name="xtp", bufs=3))
        psz = ctx.enter_context(tc.tile_pool(name="psz", bufs=2, space="PSUM"))
        pst = ctx.enter_context(tc.tile_pool(name="pst", bufs=2, space="PSUM"))
        psr = ctx.enter_context(tc.tile_pool(name="psr", bufs=2, space="PSUM"))
        psm = ctx.enter_context(tc.tile_pool(name="psm", bufs=2, space="PSUM"))

        metas = [None] * nt

        def load_meta(t):
            meta = small.tile([TILE_B, MC], BF, tag="meta")
            nc.scalar.dma_start(out=meta, in_=meta_d[t])
            metas[t] = meta

        # critical-path loads first: tile 0 x + meta
        x_tiles = [None] * nt

        def load_x(t):
            x_t = xp.tile([TILE_B, G, LX], F8, tag="x_t")
            nc.sync.dma_start(out=x_t[:, 0:G // 2, :], in_=x_d[t, :, 0:G // 2, :])
            nc.sync.dma_start(out=x_t[:, G // 2:, :], in_=x_d[t, :, G // 2:, :])
            x_tiles[t] = x_t

        load_x(0)
        load_meta(0)

        # Ping-pong block-diagonal stationaries for the PE peer-reduce.
        # s[:, g, :] is [128, 128]: column 4g+b4 holds m[g*4+b4, p] at rows
        # (b4, p); zeroed by early DMA, diagonal band rewritten per tile.
        s_tiles = [
            singles.tile([TILE_B, G, TILE_B], F8, tag=f"s{i}", name=f"s_{i}")
            for i in range(3)
        ]
        for i in range(3):
            nc.scalar.dma_start(out=s_tiles[i], in_=sz_d[:, :, :])

        wr_sb = singles.tile([TILE_B, 2, R], F32R)
        nc.sync.dma_start(out=wr_sb, in_=wr_d.rearrange("k p r -> p k r"))
        wb_sb = singles.tile([TILE_B, 2, R], BF)
        nc.sync.dma_start(out=wb_sb, in_=wb_d.rearrange("k p r -> p k r"))
        wf_sb = singles.tile([TILE_B, 2, R], F32)
        nc.sync.dma_start(out=wf_sb, in_=wf_d.rearrange("k p r -> p k r"))
        ident = singles.tile([TILE_B, TILE_B], F32)
        nc.sync.dma_start(out=ident, in_=id_d[:, :])

        def write_band(t):
            s_all = s_tiles[t % 3]
            m_t = metas[t][:, 0:G]
            for b4 in range(4):
                view = s_all[b4 * P:(b4 + 1) * P, :, :]
                out_ap = bass.AP(
                    tensor=view.tensor, offset=view.offset + b4,
                    ap=[view.ap[0], [TILE_B + 4, G]],
                )
                nc.vector.tensor_copy(
                    out=out_ap, in_=m_t[b4 * P:(b4 + 1) * P, :],
                )

        write_band(0)
        for t in range(nt):
            if t > 0:
                load_x(t)
            x_t = x_tiles[t]
            if t + 1 < nt:
                load_meta(t + 1)

            # ---- z-chain: psum_z[b, 0:256]=z, [256]=s1, [257]=s2 ----
            s_all = s_tiles[t % 3]
            psum_z = psz.tile([TILE_B, LX], F32, tag="psum_z")
            for g in range(G):
                nc.tensor.matmul(
                    out=psum_z,
                    lhsT=s_all[:, g, :],
                    rhs=x_t[:, g, :],
                    start=(g == 0),
                    stop=(g == G - 1),
                )

            # band for the next tile while PE runs this epilogue
            if t + 1 < nt:
                write_band(t + 1)

            # ---- epilogue ----
            x_sb = small.tile([TILE_B, LX], F32, tag="x_sb")
            nc.scalar.copy(out=x_sb, in_=psum_z)
            s1 = x_sb[:, L:L + 1]
            s2 = x_sb[:, L + 1:L + 2]

            meta = metas[t]
            psum_my = psm.tile([TILE_B, R], F32, tag="psum_my")
            nc.tensor.matmul(
                out=psum_my, lhsT=meta[:, G:G + TILE_B], rhs=wb_sb[:, 0, :],
                start=True, stop=False,
            )
            nc.tensor.matmul(
                out=psum_my, lhsT=meta[:, G + TILE_B:MC], rhs=wb_sb[:, 1, :],
                start=False, stop=True,
            )

            # transpose z chunks into one PSUM tile, one ACT cast to f32r
            pt = pst.tile([TILE_B, 2 * TILE_B], F32, tag="pt")
            nc.tensor.transpose(
                out=pt[:, 0:TILE_B], in_=x_sb[:, 0:TILE_B], identity=ident,
            )
            nc.tensor.transpose(
                out=pt[:, TILE_B:2 * TILE_B], in_=x_sb[:, TILE_B:2 * TILE_B],
                identity=ident,
            )
            xt_all = xtp.tile([TILE_B, 2 * TILE_B], F32R, tag="xt_all")
            nc.scalar.copy(out=xt_all, in_=pt)

            # psum_rest = z @ Wp.T/P
            psum_rest = psr.tile([TILE_B, R], F32, tag="psum_rest")
            nc.tensor.matmul(
                out=psum_rest, lhsT=xt_all[:, 0:TILE_B], rhs=wr_sb[:, 0, :],
                start=True, stop=False,
            )
            nc.tensor.matmul(
                out=psum_rest, lhsT=xt_all[:, TILE_B:2 * TILE_B],
                rhs=wr_sb[:, 1, :], start=False, stop=True,
            )

            # out = s1*psum_my + psum_rest + s1*(bias/P) + s2*(wm/P)
            # rank-1 terms on GPSIMD (SBUF-only), PSUM add on DVE
            my_sb = small.tile([TILE_B, R], F32, tag="my_sb")
            nc.scalar.mul(out=my_sb, in_=psum_my, mul=s1)
            t1 = small.tile([TILE_B, R], F32, tag="t1")
            nc.vector.scalar_tensor_tensor(
                out=t1, in0=wf_sb[:, 0, :], scalar=s1, in1=my_sb,
                op0=mybir.AluOpType.mult, op1=mybir.AluOpType.add,
            )
            t2 = small.tile([TILE_B, R], F32, tag="t2")
            nc.vector.scalar_tensor_tensor(
                out=t2, in0=wf_sb[:, 1, :], scalar=s2, in1=t1,
                op0=mybir.AluOpType.mult, op1=mybir.AluOpType.add,
            )
            out_sb = small.tile([TILE_B, R], BF, tag="out_sb")
            nc.vector.tensor_add(out_sb, t2, psum_rest)
            nc.scalar.dma_start(
                out=out_d[t * TILE_B:(t + 1) * TILE_B, :], in_=out_sb
            )

    nc.compile()
    return nc


class TileCtx:
    """with TileCtx(nc) as (tc, ctx): — TileContext plus an ExitStack."""

    def __init__(self, nc):
        from contextlib import ExitStack
        self.tc = tile.TileContext(nc)
        self.ctx = ExitStack()

    def __enter__(self):
        return self.tc.__enter__(), self.ctx.__enter__()

    def __exit__(self, *a):
        self.ctx.__exit__(*a)
        return self.tc.__exit__(*a)


def prep_inputs(my_latent, peer_latents, peer_metrics, W, b):
    """Host-side shard + layout prep (dtype casts and weight packing only)."""
    wr = np.zeros((2, TILE_B, R), dtype=np.float32)
    wpt = np.ascontiguousarray(W[:, L:2 * L].T) / P       # [256, 256] Wp.T/P
    wr[0] = wpt[0:TILE_B]
    wr[1] = wpt[TILE_B:2 * TILE_B]
    wb = np.zeros((2, TILE_B, R), dtype=ml_dtypes.bfloat16)
    wmt = np.ascontiguousarray(W[:, 0:L].T) / P           # [256, 256] Wm.T/P
    wb[0] = wmt[0:TILE_B].astype(ml_dtypes.bfloat16)
    wb[1] = wmt[TILE_B:2 * TILE_B].astype(ml_dtypes.bfloat16)
    wf = np.empty((2, TILE_B, R), dtype=np.float32)
    wf[0] = np.broadcast_to(b / P, (TILE_B, R))           # pairs with s1
    wf[1] = np.broadcast_to(W[:, 2 * L] / P, (TILE_B, R))  # wm/P, with s2
    ident = np.eye(TILE_B, dtype=np.float32)
    sz = np.zeros((TILE_B, G, TILE_B), dtype=ml_dtypes.float8_e3m4)

    in_maps = []
    for c in range(N_CORES):
        sl = slice(c * BC, (c + 1) * BC)
        # PE band layout [(b4,p), g, l] + the two extra moving cols
        perm = peer_latents[sl].reshape(NT, G, 4, P, L).transpose(
            0, 2, 3, 1, 4)                                # [NT, 4, P, G, L]
        mt = peer_metrics[sl].reshape(NT, G, 4, P).transpose(
            0, 2, 3, 1).reshape(NT, TILE_B, G)            # [NT, (b4,p), G]
        xc = np.empty((NT, TILE_B, G, LX), dtype=ml_dtypes.float8_e3m4)
        xc[:, :, :, 0:L] = perm.reshape(NT, TILE_B, G, L).astype(
            ml_dtypes.float8_e3m4)
        xc[:, :, :, L] = ml_dtypes.float8_e3m4(1.0)
        xc[:, :, :, L + 1] = mt.astype(ml_dtypes.float8_e3m4)

        meta = np.empty((NT, TILE_B, MC), dtype=ml_dtypes.bfloat16)
        meta[:, :, 0:G] = mt.astype(ml_dtypes.bfloat16)
        myt = my_latent[sl].reshape(NT, TILE_B, L).transpose(0, 2, 1).astype(
            ml_dtypes.bfloat16)                           # [NT, l, b]
        meta[:, :, G:G + TILE_B] = myt[:, 0:TILE_B, :]
        meta[:, :, G + TILE_B:] = myt[:, TILE_B:2 * TILE_B, :]
        in_maps.append({
            "x": xc,
            "meta": meta,
            "wr": wr,
            "wb": wb,
            "wf": wf,
            "ident": ident,
            "sz": sz,
        })
    return in_maps


def run(my_latent, peer_latents, peer_metrics, W, b, trace=False, **kw):
    if "nc" not in _cache:
        _cache["nc"] = build_bass()
    nc = _cache["nc"]
    in_maps = prep_inputs(
        np.asarray(my_latent, dtype=np.float32),
        np.asarray(peer_latents, dtype=np.float32),
        np.asarray(peer_metrics, dtype=np.float32),
        np.asarray(W, dtype=np.float32),
        np.asarray(b, dtype=np.float32),
    )
    res = run_bass_kernel_spmd(
        nc, in_maps, core_ids=list(range(N_CORES)), trace=trace, **kw
    )
    out = np.concatenate(
        [np.asarray(r["out"]).astype(np.float32) for r in res.results], axis=0
    )
    return out, res


def kernel(my_latent, peer_latents, peer_metrics, W, b):
    out, _ = run(my_latent, peer_latents, peer_metrics, W, b)
    return out
